# revision 39
# baseline (speedup 1.0000x reference)
"""Trainium2 Bass kernel for nn_ExpertFFN (top-1 MoE, B=4 S=2048 H=1024 E=8).

Strategy: shard tokens (batch*seq = 8192) across 8 NeuronCores, 1024 tokens
per core; replicate router and all 8 expert weights on every core.  Per core:

  1. load x token-major, PE-transpose to feature-major X^T (fp32)
  2. fp32 router matmul + softmax (top-1 gate = 1/sum(exp(l - max)), onehot
     via is_equal against the row max)
  3. slot assignment in one PSUM pass + small DVE prefix:
       slot(t) = cumsum_tile(t,e) - 1 + tile_base(tile,e) + 176*e  @ e=argmax
  4. one batched scatter of token-ids by slot into a DRAM index table
     (inverse permutation), sentinel 9999 in empty slots; when expert bias is
     zero the gate is folded into x (y = (g*x) @ W) and the scaled x is
     written to DRAM staging for the dispatch gathers
  5. per expert e: indirect-gather its <=176 token rows, split hi/lo bf16,
     PE-transpose (bf16), grouped GEMM as 3-term bf16 decomposition
       x*w ~= x_hi*w_hi + x_lo*w_hi + x_hi*w_lo   (fp32 PSUM accumulation)
     with weights pre-split hi/lo on host, fp32 PE-transpose back to
     token-major, indirect-scatter rows to y (bounds_check skips empty slots)

Expert weights stream on the sync DMA queue ahead of everything
index-dependent; index/gate traffic uses the scalar HWDGE queue so weight
prefetch is never head-of-line blocked.
"""

import os
import sys

for _p in ("/opt/trn_rl_repo",):
    if _p not in sys.path:
        sys.path.insert(0, _p)

import numpy as np

P = 128
H = 1024
E = 8
TPC = 1024          # tokens per core
NCORES = 8
KC = H // P         # contraction chunks
MC = H // P         # output feature chunks
NTT = TPC // P      # token tiles per core
CAP = 176           # per-expert slot capacity (max observed group 172)
CAPA, CAPB = 128, CAP - 128
NSLOT = E * CAP     # 1408
SENTINEL = 9999
PREC = os.environ.get("MOE_PREC", "hilo3")   # hilo3 | hilo4 | fp32


def _build(router_bias: bool, expert_bias: bool, prec: str = PREC):
    import concourse.bass as bass
    import concourse.mybir as mybir
    import concourse.tile as tile
    from concourse import bacc
    from concourse.masks import make_identity, make_upper_triangular

    f32 = mybir.dt.float32
    bf16 = mybir.dt.bfloat16
    i32 = mybir.dt.int32
    AX = mybir.AxisListType
    OP = mybir.AluOpType
    ACT = mybir.ActivationFunctionType
    hilo = prec.startswith("hilo")
    four_term = prec == "hilo4"
    # gate folded into x unless the expert bias path needs post-scaling
    prescale = not expert_bias

    nc = bacc.Bacc("TRN2", target_bir_lowering=False, debug=False,
                   num_devices=NCORES)

    x_d = nc.dram_tensor("x", [TPC, H], f32, kind="ExternalInput")
    rw_d = nc.dram_tensor("router_w", [H, E], f32, kind="ExternalInput")
    rb_d = nc.dram_tensor("router_b", [E], f32, kind="ExternalInput")
    if hilo:
        ewh_d = nc.dram_tensor("ew_hi", [E, H, H], bf16, kind="ExternalInput")
        ewl_d = nc.dram_tensor("ew_lo", [E, H, H], bf16, kind="ExternalInput")
    else:
        ew_d = nc.dram_tensor("expert_w", [E, H, H], f32,
                              kind="ExternalInput")
    eb_d = nc.dram_tensor("expert_b", [E, H], f32, kind="ExternalInput")
    y_d = nc.dram_tensor("y", [TPC, H], f32, kind="ExternalOutput")

    with tile.TileContext(nc) as tc:
        with (
            tc.tile_pool(name="consts", bufs=1) as cpool,
            tc.tile_pool(name="dram", bufs=1, space="DRAM") as dpool,
            tc.tile_pool(name="wload", bufs=2 * KC) as wpool,
        ):
            # constants
            id128 = cpool.tile([P, P], f32)
            make_identity(nc, id128[:])
            if hilo:
                idb = cpool.tile([P, P], bf16)
                make_identity(nc, idb[:])
            lt128 = cpool.tile([P, P], f32)
            make_upper_triangular(nc, lt128[:], val=1.0, diag=True)
            ones_1x = cpool.tile([1, P], f32)
            nc.gpsimd.memset(ones_1x[:], 1.0)
            ones128 = cpool.tile([P, P], f32)
            nc.gpsimd.memset(ones128[:], 1.0)
            ones_cap = cpool.tile([1, CAP], f32)
            nc.gpsimd.memset(ones_cap[:], 1.0)
            ecm1_i = cpool.tile([P, E], i32)
            nc.gpsimd.iota(ecm1_i[:], pattern=[[CAP, E]], base=-1,
                           channel_multiplier=0)
            # token ids: tid_all[p, j] = j*128 + p
            tid_all = cpool.tile([P, NTT], i32)
            nc.gpsimd.iota(tid_all[:], pattern=[[P, NTT]], base=0,
                           channel_multiplier=1)
            sent = cpool.tile([1, NSLOT], i32)
            nc.gpsimd.memset(sent[:], SENTINEL)

            # DRAM scratch (pool tiles so Tile tracks cross-phase deps).
            # Everything touching gidx_dram stays on the gpsimd queue so the
            # prefill -> scatter -> readback chain is engine-FIFO ordered.
            gidx_dram = dpool.tile([NSLOT, 1], i32)
            nc.gpsimd.dma_start(out=gidx_dram[:], in_=sent[:])
            # dispatch target: x rows forward-scattered into slot order
            xs_dram = dpool.tile([NSLOT, H], f32)
            if not prescale:
                gate_dram = dpool.tile([TPC, 1], f32)

            # expert weights stream on the sync queue; experts 0-1 prefetch
            # up front, the rest are emitted two experts ahead inside the
            # main loop so sync stays available for staging loads
            def load_w(e):
                if hilo:
                    whs, wls = [], []
                    for k in range(KC):
                        whk = wpool.tile([P, H], bf16, tag="wh")
                        nc.sync.dma_start(
                            out=whk[:], in_=ewh_d[e, k * P:(k + 1) * P, :])
                        whs.append(whk)
                        wlk = wpool.tile([P, H], bf16, tag="wl")
                        nc.sync.dma_start(
                            out=wlk[:], in_=ewl_d[e, k * P:(k + 1) * P, :])
                        wls.append(wlk)
                    return (whs, wls)
                ws = []
                for k in range(KC):
                    wk = wpool.tile([P, H], f32, tag="w")
                    nc.sync.dma_start(
                        out=wk[:], in_=ew_d[e, k * P:(k + 1) * P, :])
                    ws.append(wk)
                return ws

            w_tiles = {e: load_w(e) for e in range(2)}

            # ---------------- phase 1: router + slot assignment ----------
            with (
                tc.tile_pool(name="rsb", bufs=NTT) as rpool,
                tc.tile_pool(name="rsmall", bufs=NTT) as spool,
                tc.tile_pool(name="rps", bufs=2, space="PSUM") as rpsum,
                tc.tile_pool(name="cps", bufs=2, space="PSUM") as cpsum,
                tc.tile_pool(name="cps1", bufs=2, space="PSUM") as cpsum1,
            ):
                xtm = []
                for t in range(NTT):
                    xt = rpool.tile([P, H], f32, tag="xtm")
                    eng = nc.scalar if t % 2 == 0 else nc.gpsimd
                    eng.dma_start(out=xt[:], in_=x_d[t * P:(t + 1) * P, :])
                    xtm.append(xt)
                rw_sb = []
                for k in range(KC):
                    rwk = spool.tile([P, E], f32, tag="rw")
                    nc.scalar.dma_start(out=rwk[:],
                                        in_=rw_d[k * P:(k + 1) * P, :])
                    rw_sb.append(rwk)
                if router_bias:
                    rb_sb = spool.tile([1, E], f32, tag="rb")
                    nc.scalar.dma_start(out=rb_sb[:], in_=rb_d[None, :])

                # per token tile: X^T (k along free dim), logits, softmax,
                # onehot -- tile-granular so each tile's router matmuls fire
                # as soon as its own 8 transposes land
                oh_all = rpool.tile([P, NTT * E], f32, tag="ohall")
                gate = []
                xsc = []
                for t in range(NTT):
                    pxt = rpsum.tile([P, H], f32, tag="pxt", space="PSUM")
                    for k in range(KC):
                        nc.tensor.transpose(
                            out=pxt[:, k * P:(k + 1) * P],
                            in_=xtm[t][:, k * P:(k + 1) * P],
                            identity=id128[:])
                    xTt = rpool.tile([P, H], f32, tag="xTt")
                    nc.vector.tensor_copy(out=xTt[:], in_=pxt[:])

                    plg = cpsum.tile([P, E], f32, tag="plg", space="PSUM")
                    for k in range(KC):
                        nc.tensor.matmul(
                            out=plg[:], lhsT=xTt[:, k * P:(k + 1) * P],
                            rhs=rw_sb[k][:], start=(k == 0),
                            stop=(k == KC - 1 and not router_bias))
                    if router_bias:
                        nc.tensor.matmul(out=plg[:], lhsT=ones_1x[:],
                                         rhs=rb_sb[:], start=False, stop=True)

                    negm = spool.tile([P, 1], f32, tag="negm")
                    nc.vector.tensor_reduce(out=negm[:], in_=plg[:], axis=AX.X,
                                            op=OP.max, negate=True)
                    m_t = spool.tile([P, 1], f32, tag="m")
                    nc.vector.tensor_scalar_mul(out=m_t[:], in0=negm[:],
                                                scalar1=-1.0)
                    esum = spool.tile([P, 1], f32, tag="esum")
                    etmp = spool.tile([P, E], f32, tag="etmp")
                    nc.scalar.activation(out=etmp[:], in_=plg[:], func=ACT.Exp,
                                         bias=negm[:], scale=1.0,
                                         accum_out=esum[:])
                    g_t = spool.tile([P, 1], f32, tag="gate")
                    nc.vector.reciprocal(out=g_t[:], in_=esum[:])
                    gate.append(g_t)
                    nc.vector.tensor_scalar(
                        out=oh_all[:, t * E:(t + 1) * E], in0=plg[:],
                        scalar1=m_t[:], scalar2=None, op0=OP.is_equal)
                    if prescale:
                        # fold gate into x; scattered to slot order below
                        xs_t = rpool.tile([P, H], f32, tag="xsc")
                        nc.vector.tensor_scalar(out=xs_t[:], in0=xtm[t][:],
                                                scalar1=g_t[:], scalar2=None,
                                                op0=OP.mult)
                        xsc.append(xs_t)
                    else:
                        xsc.append(xtm[t])
                        nc.scalar.dma_start(
                            out=gate_dram[t * P:(t + 1) * P, :], in_=g_t[:])

                # per-tile incremental slot computation so tile t's dispatch
                # scatter fires as soon as its softmax + running prefix land:
                #   slot(t) = (cumsum_t - 1 + tile_base_t + e*CAP) . onehot_t
                tbacc = spool.tile([P, E], f32, tag="tbacc")
                nc.vector.tensor_copy(out=tbacc[:], in_=ecm1_i[:])
                slot_is = []
                for t in range(NTT):
                    blk = slice(t * E, (t + 1) * E)
                    # [cumsum_t | count_bcast_t] in one PSUM tile
                    pcc = cpsum1.tile([P, 2 * E], f32, tag="pcc",
                                      space="PSUM")
                    nc.tensor.matmul(out=pcc[:, 0:E], lhsT=lt128[:],
                                     rhs=oh_all[:, blk], start=True,
                                     stop=True)
                    nc.tensor.matmul(out=pcc[:, E:2 * E], lhsT=ones128[:],
                                     rhs=oh_all[:, blk], start=True,
                                     stop=True)
                    tmp = spool.tile([P, E], f32, tag="tmp")
                    nc.vector.tensor_tensor(out=tmp[:], in0=pcc[:, 0:E],
                                            in1=tbacc[:], op=OP.add)
                    junk = spool.tile([P, E], f32, tag="junk")
                    nc.vector.tensor_tensor(out=junk[:], in0=tmp[:],
                                            in1=oh_all[:, blk], op=OP.mult)
                    slot_f = spool.tile([P, 1], f32, tag="slotf")
                    nc.vector.tensor_reduce(out=slot_f[:], in_=junk[:],
                                            axis=AX.X, op=OP.add)
                    slot_i = spool.tile([P, 1], i32, tag="sloti")
                    nc.vector.tensor_copy(out=slot_i[:], in_=slot_f[:])
                    slot_is.append(slot_i)
                    if t < NTT - 1:
                        nc.vector.tensor_tensor(out=tbacc[:], in0=tbacc[:],
                                                in1=pcc[:, E:2 * E],
                                                op=OP.add)
                    # dispatch: forward-scatter (scaled) x rows to slot order
                    nc.gpsimd.indirect_dma_start(
                        out=xs_dram[:],
                        out_offset=bass.IndirectOffsetOnAxis(
                            ap=slot_i[:, :1], axis=0),
                        in_=xsc[t][:], in_offset=None)
                # inverse permutation (combine-time): gidx[slot] = token
                for t in range(NTT):
                    nc.gpsimd.indirect_dma_start(
                        out=gidx_dram[:],
                        out_offset=bass.IndirectOffsetOnAxis(
                            ap=slot_is[t][:, :1], axis=0),
                        in_=tid_all[:, t:t + 1], in_offset=None)

            # ---------------- phase 2: per-expert grouped GEMM ------------
            with (
                tc.tile_pool(name="est", bufs=3) as stpool,
                tc.tile_pool(name="exs", bufs=2 * KC) as xspool,
                tc.tile_pool(name="eyt", bufs=2 * MC) as ytpool,
                tc.tile_pool(name="eysb", bufs=3) as ypool,
                tc.tile_pool(name="egi", bufs=E) as gipool,
                tc.tile_pool(name="exps", bufs=2, space="PSUM") as xpsum,
                tc.tile_pool(name="eyps", bufs=2, space="PSUM") as ypsum,
                tc.tile_pool(name="etps", bufs=2, space="PSUM") as tpsum,
            ):
                # front-load all index readbacks (gpsimd queue, after scatters)
                gAB = []
                for e in range(E):
                    base = e * CAP
                    gA = gipool.tile([CAPA, 1], i32, tag="gA")
                    nc.gpsimd.dma_start(out=gA[:],
                                        in_=gidx_dram[base:base + CAPA, :])
                    gB = gipool.tile([CAPB, 1], i32, tag="gB")
                    nc.gpsimd.dma_start(
                        out=gB[:], in_=gidx_dram[base + CAPA:base + CAP, :])
                    gAB.append((gA, gB))

                for e in range(E):
                    gA, gB = gAB[e]
                    base = e * CAP
                    # staging is already in slot order: plain loads, spread
                    # over the scalar/sync/gpsimd queues
                    stA = stpool.tile([CAPA, H], f32, tag="stA")
                    engA = nc.scalar if e % 2 == 0 else nc.sync
                    engA.dma_start(out=stA[:],
                                   in_=xs_dram[base:base + CAPA, :])
                    stB = stpool.tile([CAPB, H], f32, tag="stB")
                    nc.gpsimd.dma_start(
                        out=stB[:], in_=xs_dram[base + CAPA:base + CAP, :])
                    if e + 2 < E:
                        w_tiles[e + 2] = load_w(e + 2)

                    if hilo:
                        stAh = stpool.tile([CAPA, H], bf16, tag="stAh")
                        nc.vector.tensor_copy(out=stAh[:], in_=stA[:])
                        stAhf = stpool.tile([CAPA, H], f32, tag="stAhf")
                        nc.vector.tensor_copy(out=stAhf[:], in_=stAh[:])
                        stAl = stpool.tile([CAPA, H], bf16, tag="stAl")
                        nc.vector.tensor_tensor(out=stAl[:], in0=stA[:],
                                                in1=stAhf[:], op=OP.subtract)
                        stBh = stpool.tile([CAPB, H], bf16, tag="stBh")
                        nc.vector.tensor_copy(out=stBh[:], in_=stB[:])
                        stBhf = stpool.tile([CAPB, H], f32, tag="stBhf")
                        nc.vector.tensor_copy(out=stBhf[:], in_=stBh[:])
                        stBl = stpool.tile([CAPB, H], bf16, tag="stBl")
                        nc.vector.tensor_tensor(out=stBl[:], in0=stB[:],
                                                in1=stBhf[:], op=OP.subtract)

                        xsh, xsl = [], []
                        for k in range(KC):
                            ks = slice(k * P, (k + 1) * P)
                            pxh = xpsum.tile([P, CAP], bf16, tag="pxs",
                                             space="PSUM")
                            nc.tensor.transpose(out=pxh[:, 0:CAPA],
                                                in_=stAh[:, ks],
                                                identity=idb[:])
                            nc.tensor.transpose(out=pxh[:, CAPA:CAP],
                                                in_=stBh[:, ks],
                                                identity=idb[:CAPB, :CAPB])
                            xshk = xspool.tile([P, CAP], bf16, tag="xsh")
                            nc.vector.tensor_copy(out=xshk[:], in_=pxh[:])
                            xsh.append(xshk)
                            pxl = xpsum.tile([P, CAP], bf16, tag="pxs",
                                             space="PSUM")
                            nc.tensor.transpose(out=pxl[:, 0:CAPA],
                                                in_=stAl[:, ks],
                                                identity=idb[:])
                            nc.tensor.transpose(out=pxl[:, CAPA:CAP],
                                                in_=stBl[:, ks],
                                                identity=idb[:CAPB, :CAPB])
                            xslk = xspool.tile([P, CAP], bf16, tag="xsl")
                            nc.vector.tensor_copy(out=xslk[:], in_=pxl[:])
                            xsl.append(xslk)
                        wh_sb, wl_sb = w_tiles[e]
                    else:
                        xs = []
                        for k in range(KC):
                            ks = slice(k * P, (k + 1) * P)
                            pxs = xpsum.tile([P, CAP], f32, tag="pxs",
                                             space="PSUM")
                            nc.tensor.transpose(out=pxs[:, 0:CAPA],
                                                in_=stA[:, ks],
                                                identity=id128[:])
                            nc.tensor.transpose(out=pxs[:, CAPA:CAP],
                                                in_=stB[:, ks],
                                                identity=id128[:CAPB, :CAPB])
                            xsk = xspool.tile([P, CAP], f32, tag="xs")
                            nc.vector.tensor_copy(out=xsk[:], in_=pxs[:])
                            xs.append(xsk)
                        w_sb = w_tiles[e]

                    if expert_bias:
                        eb_sb = gipool.tile([1, H], f32, tag="eb")
                        nc.scalar.dma_start(out=eb_sb[:], in_=eb_d[e, None, :])

                    # grouped GEMM: Y^T[m] = sum_k W[k,m]^T X^T[k]  (+ b)
                    yt = []
                    for m in range(MC):
                        ms = slice(m * P, (m + 1) * P)
                        pyt = ypsum.tile([P, CAP], f32, tag="pyt",
                                         space="PSUM")
                        if hilo:
                            for k in range(KC):
                                last = (k == KC - 1 and not expert_bias)
                                nc.tensor.matmul(
                                    out=pyt[:], lhsT=wh_sb[k][:, ms],
                                    rhs=xsh[k][:], start=(k == 0), stop=False)
                                nc.tensor.matmul(
                                    out=pyt[:], lhsT=wh_sb[k][:, ms],
                                    rhs=xsl[k][:], start=False, stop=False)
                                nc.tensor.matmul(
                                    out=pyt[:], lhsT=wl_sb[k][:, ms],
                                    rhs=xsh[k][:], start=False,
                                    stop=(last and not four_term))
                                if four_term:
                                    nc.tensor.matmul(
                                        out=pyt[:], lhsT=wl_sb[k][:, ms],
                                        rhs=xsl[k][:], start=False, stop=last)
                        else:
                            for k in range(KC):
                                nc.tensor.matmul(
                                    out=pyt[:], lhsT=w_sb[k][:, ms],
                                    rhs=xs[k][:], start=(k == 0),
                                    stop=(k == KC - 1 and not expert_bias))
                        if expert_bias:
                            nc.tensor.matmul(
                                out=pyt[:], lhsT=eb_sb[:, ms],
                                rhs=ones_cap[:], start=False, stop=True)
                        ytm = ytpool.tile([P, CAP], f32, tag="yt")
                        nc.vector.tensor_copy(out=ytm[:], in_=pyt[:])
                        yt.append(ytm)

                    if not prescale:
                        gsA = gipool.tile([CAPA, 1], f32, tag="gsA")
                        nc.gpsimd.indirect_dma_start(
                            out=gsA[:], out_offset=None, in_=gate_dram[:],
                            in_offset=bass.IndirectOffsetOnAxis(ap=gA[:, :1],
                                                                axis=0),
                            bounds_check=TPC - 1, oob_is_err=False)
                        gsB = gipool.tile([CAPB, 1], f32, tag="gsB")
                        nc.gpsimd.indirect_dma_start(
                            out=gsB[:], out_offset=None, in_=gate_dram[:],
                            in_offset=bass.IndirectOffsetOnAxis(ap=gB[:, :1],
                                                                axis=0),
                            bounds_check=TPC - 1, oob_is_err=False)

                    # fp32 transpose back to token-major, scatter rows to y
                    ptokA = tpsum.tile([P, H], f32, tag="ptok", space="PSUM")
                    for m in range(MC):
                        ms = slice(m * P, (m + 1) * P)
                        nc.tensor.transpose(out=ptokA[:, ms],
                                            in_=yt[m][:, 0:CAPA],
                                            identity=id128[:])
                    yA = ypool.tile([CAPA, H], f32, tag="yA")
                    if prescale:
                        nc.vector.tensor_copy(out=yA[:], in_=ptokA[:])
                    else:
                        nc.vector.tensor_scalar(out=yA[:], in0=ptokA[:],
                                                scalar1=gsA[:], scalar2=None,
                                                op0=OP.mult)
                    nc.gpsimd.indirect_dma_start(
                        out=y_d[:],
                        out_offset=bass.IndirectOffsetOnAxis(ap=gA[:, :1],
                                                            axis=0),
                        in_=yA[:], in_offset=None,
                        bounds_check=TPC - 1, oob_is_err=False)

                    ptokB = tpsum.tile([P, H], f32, tag="ptok", space="PSUM")
                    for m in range(MC):
                        ms = slice(m * P, (m + 1) * P)
                        nc.tensor.transpose(out=ptokB[0:CAPB, ms],
                                            in_=yt[m][:, CAPA:CAP],
                                            identity=id128[:])
                    yB = ypool.tile([CAPB, H], f32, tag="yB")
                    if prescale:
                        nc.vector.tensor_copy(out=yB[:], in_=ptokB[0:CAPB, :])
                    else:
                        nc.vector.tensor_scalar(out=yB[:], in0=ptokB[0:CAPB, :],
                                                scalar1=gsB[:], scalar2=None,
                                                op0=OP.mult)
                    nc.gpsimd.indirect_dma_start(
                        out=y_d[:],
                        out_offset=bass.IndirectOffsetOnAxis(ap=gB[:, :1],
                                                            axis=0),
                        in_=yB[:], in_offset=None,
                        bounds_check=TPC - 1, oob_is_err=False)

    nc.compile()
    return nc


_NC_CACHE = {}


def _get_nc(router_bias: bool, expert_bias: bool, prec: str = PREC):
    key = (router_bias, expert_bias, prec)
    if key not in _NC_CACHE:
        _NC_CACHE[key] = _build(*key)
    return _NC_CACHE[key]


def _split_hilo(w):
    import ml_dtypes
    hi = w.astype(ml_dtypes.bfloat16)
    lo = (w - hi.astype(np.float32)).astype(ml_dtypes.bfloat16)
    return np.ascontiguousarray(hi), np.ascontiguousarray(lo)


def make_in_maps(x, router_w, router_b, expert_w, expert_b, prec=PREC):
    xt = x.reshape(NCORES, TPC, H)
    base = {"router_w": router_w, "router_b": router_b, "expert_b": expert_b}
    if prec.startswith("hilo"):
        hi, lo = _split_hilo(expert_w)
        base["ew_hi"] = hi
        base["ew_lo"] = lo
    else:
        base["expert_w"] = expert_w
    return [dict(base, x=np.ascontiguousarray(xt[c])) for c in range(NCORES)]


def kernel(x, router_w, router_b, expert_w, expert_b):
    from concourse.bass_utils import run_bass_kernel_spmd

    x = np.ascontiguousarray(np.asarray(x, dtype=np.float32))
    router_w = np.ascontiguousarray(np.asarray(router_w, dtype=np.float32))
    router_b = np.ascontiguousarray(np.asarray(router_b, dtype=np.float32))
    expert_w = np.ascontiguousarray(np.asarray(expert_w, dtype=np.float32))
    expert_b = np.ascontiguousarray(np.asarray(expert_b, dtype=np.float32))

    B, S, Hx = x.shape
    assert (B * S, Hx) == (NCORES * TPC, H), (x.shape,)

    # host-side safety: capacity must hold for these inputs
    logits = x.reshape(-1, H) @ router_w + router_b
    eidx = logits.argmax(-1).reshape(NCORES, TPC)
    for c in range(NCORES):
        cnts = np.bincount(eidx[c], minlength=E)
        assert cnts.max() <= CAP, (
            f"expert capacity {CAP} exceeded on core {c}: {cnts}")

    router_bias = bool(np.any(router_b != 0))
    expert_bias = bool(np.any(expert_b != 0))
    nc = _get_nc(router_bias, expert_bias)

    in_maps = make_in_maps(x, router_w, router_b, expert_w, expert_b)
    res = run_bass_kernel_spmd(nc, in_maps, list(range(NCORES)))
    y = np.concatenate([res.results[c]["y"] for c in range(NCORES)], axis=0)
    return y.reshape(B, S, H)


# revision 40
# speedup vs baseline: 1.0674x; 1.0674x over previous
"""Trainium2 Bass kernel for nn_ExpertFFN (top-1 MoE, B=4 S=2048 H=1024 E=8).

Strategy: shard tokens (batch*seq = 8192) across 8 NeuronCores, 1024 tokens
per core, with a load-balancing shard: each expert's tokens are dealt
round-robin across cores so every (core, expert) group is ~n_e/8 and the
static per-expert capacity can be small.  Router and expert weights are
replicated.  Per core:

  1. load x token-major, PE-transpose to feature-major (per token tile)
  2. fp32 router matmul + softmax (top-1 gate = 1/sum(exp(l - max)), onehot
     via is_equal against the row max)
  3. per-tile incremental slot assignment (PSUM cumsum matmuls + DVE prefix):
       slot(t) = cumsum_tile(t,e) - 1 + tile_base(tile,e) + CAP*e  @ e=argmax
  4. forward-scatter the gate-scaled x rows into a slot-ordered DRAM buffer
     (dispatch), scatter token ids into a DRAM inverse-permutation table
     (combine-time), sentinel 9999 in empty slots
  5. per expert e: contiguous staging loads, fp32 PE-transpose to
     feature-major, hi/lo bf16 split on DVE, grouped GEMM as 3-term bf16
     decomposition   x*w ~= x_hi*w_hi + x_lo*w_hi + x_hi*w_lo   with fp32
     PSUM accumulation and host-pre-split weights, fp32 PE-transpose back
     to token-major, indirect-scatter rows to y (bounds skips empty slots)

Expert weights stream on the sync DMA queue; index/staging traffic uses the
scalar/gpsimd queues so weight prefetch is never head-of-line blocked.
"""

import os
import sys

for _p in ("/opt/trn_rl_repo",):
    if _p not in sys.path:
        sys.path.insert(0, _p)

import numpy as np

P = 128
H = 1024
E = 8
TPC = 1024          # tokens per core
NCORES = 8
KC = H // P         # contraction chunks
MC = H // P         # output feature chunks
NTT = TPC // P      # token tiles per core
SENTINEL = 9999
PREC = os.environ.get("MOE_PREC", "hilo3")   # hilo3 | hilo4 | fp32
DEFAULT_CAP = 136   # balanced shard keeps every (core, expert) group <= this


def _build(router_bias: bool, expert_bias: bool, cap: int = DEFAULT_CAP,
           prec: str = PREC):
    import concourse.bass as bass
    import concourse.mybir as mybir
    import concourse.tile as tile
    from concourse import bacc
    from concourse.masks import make_identity, make_upper_triangular

    f32 = mybir.dt.float32
    bf16 = mybir.dt.bfloat16
    i32 = mybir.dt.int32
    AX = mybir.AxisListType
    OP = mybir.AluOpType
    ACT = mybir.ActivationFunctionType
    hilo = prec.startswith("hilo")
    four_term = prec == "hilo4"
    prescale = not expert_bias
    CAP = cap
    CAPA, CAPB = P, CAP - P
    NSLOT = E * CAP

    nc = bacc.Bacc("TRN2", target_bir_lowering=False, debug=False,
                   num_devices=NCORES)

    x_d = nc.dram_tensor("x", [TPC, H], f32, kind="ExternalInput")
    rw_d = nc.dram_tensor("router_w", [H, E], f32, kind="ExternalInput")
    rb_d = nc.dram_tensor("router_b", [E], f32, kind="ExternalInput")
    if hilo:
        ewh_d = nc.dram_tensor("ew_hi", [E, H, H], bf16, kind="ExternalInput")
        ewl_d = nc.dram_tensor("ew_lo", [E, H, H], bf16, kind="ExternalInput")
    else:
        ew_d = nc.dram_tensor("expert_w", [E, H, H], f32,
                              kind="ExternalInput")
    eb_d = nc.dram_tensor("expert_b", [E, H], f32, kind="ExternalInput")
    y_d = nc.dram_tensor("y", [TPC, H], f32, kind="ExternalOutput")

    with tile.TileContext(nc) as tc:
        with (
            tc.tile_pool(name="consts", bufs=1) as cpool,
            tc.tile_pool(name="dram", bufs=1, space="DRAM") as dpool,
            tc.tile_pool(name="wload", bufs=2 * KC) as wpool,
        ):
            id128 = cpool.tile([P, P], f32)
            make_identity(nc, id128[:])
            lt128 = cpool.tile([P, P], f32)
            make_upper_triangular(nc, lt128[:], val=1.0, diag=True)
            ones_1x = cpool.tile([1, P], f32)
            nc.gpsimd.memset(ones_1x[:], 1.0)
            ones128 = cpool.tile([P, P], f32)
            nc.gpsimd.memset(ones128[:], 1.0)
            ones_cap = cpool.tile([1, CAP], f32)
            nc.gpsimd.memset(ones_cap[:], 1.0)
            ecm1_i = cpool.tile([P, E], i32)
            nc.gpsimd.iota(ecm1_i[:], pattern=[[CAP, E]], base=-1,
                           channel_multiplier=0)
            tid_all = cpool.tile([P, NTT], i32)
            nc.gpsimd.iota(tid_all[:], pattern=[[P, NTT]], base=0,
                           channel_multiplier=1)
            sent = cpool.tile([1, NSLOT], i32)
            nc.gpsimd.memset(sent[:], SENTINEL)

            gidx_dram = dpool.tile([NSLOT, 1], i32)
            nc.gpsimd.dma_start(out=gidx_dram[:], in_=sent[:])
            xs_dram = dpool.tile([NSLOT, H], f32)
            if not prescale:
                gate_dram = dpool.tile([TPC, 1], f32)

            # expert weights on the sync queue: experts 0-1 up front, rest
            # two experts ahead inside the main loop
            def load_w(e):
                if hilo:
                    whs, wls = [], []
                    for k in range(KC):
                        whk = wpool.tile([P, H], bf16, tag="wh")
                        nc.sync.dma_start(
                            out=whk[:], in_=ewh_d[e, k * P:(k + 1) * P, :])
                        whs.append(whk)
                        wlk = wpool.tile([P, H], bf16, tag="wl")
                        nc.sync.dma_start(
                            out=wlk[:], in_=ewl_d[e, k * P:(k + 1) * P, :])
                        wls.append(wlk)
                    return (whs, wls)
                ws = []
                for k in range(KC):
                    wk = wpool.tile([P, H], f32, tag="w")
                    nc.sync.dma_start(
                        out=wk[:], in_=ew_d[e, k * P:(k + 1) * P, :])
                    ws.append(wk)
                return ws

            w_tiles = {e: load_w(e) for e in range(2)}

            # ---------------- phase 1: router + slot assignment ----------
            with (
                tc.tile_pool(name="rsb", bufs=NTT) as rpool,
                tc.tile_pool(name="rsmall", bufs=NTT) as spool,
                tc.tile_pool(name="rps", bufs=2, space="PSUM") as rpsum,
                tc.tile_pool(name="cps", bufs=2, space="PSUM") as cpsum,
                tc.tile_pool(name="cps1", bufs=2, space="PSUM") as cpsum1,
            ):
                xtm = []
                for t in range(NTT):
                    xt = rpool.tile([P, H], f32, tag="xtm")
                    eng = nc.scalar if t % 2 == 0 else nc.gpsimd
                    eng.dma_start(out=xt[:], in_=x_d[t * P:(t + 1) * P, :])
                    xtm.append(xt)
                rw_sb = []
                for k in range(KC):
                    rwk = spool.tile([P, E], f32, tag="rw")
                    nc.scalar.dma_start(out=rwk[:],
                                        in_=rw_d[k * P:(k + 1) * P, :])
                    rw_sb.append(rwk)
                if router_bias:
                    rb_sb = spool.tile([1, E], f32, tag="rb")
                    nc.scalar.dma_start(out=rb_sb[:], in_=rb_d[None, :])

                oh_all = rpool.tile([P, NTT * E], f32, tag="ohall")
                gate = []
                xsc = []
                for t in range(NTT):
                    pxt = rpsum.tile([P, H], f32, tag="pxt", space="PSUM")
                    for k in range(KC):
                        nc.tensor.transpose(
                            out=pxt[:, k * P:(k + 1) * P],
                            in_=xtm[t][:, k * P:(k + 1) * P],
                            identity=id128[:])
                    xTt = rpool.tile([P, H], f32, tag="xTt")
                    nc.vector.tensor_copy(out=xTt[:], in_=pxt[:])

                    plg = cpsum.tile([P, E], f32, tag="plg", space="PSUM")
                    for k in range(KC):
                        nc.tensor.matmul(
                            out=plg[:], lhsT=xTt[:, k * P:(k + 1) * P],
                            rhs=rw_sb[k][:], start=(k == 0),
                            stop=(k == KC - 1 and not router_bias))
                    if router_bias:
                        nc.tensor.matmul(out=plg[:], lhsT=ones_1x[:],
                                         rhs=rb_sb[:], start=False, stop=True)

                    negm = spool.tile([P, 1], f32, tag="negm")
                    nc.vector.tensor_reduce(out=negm[:], in_=plg[:], axis=AX.X,
                                            op=OP.max, negate=True)
                    m_t = spool.tile([P, 1], f32, tag="m")
                    nc.vector.tensor_scalar_mul(out=m_t[:], in0=negm[:],
                                                scalar1=-1.0)
                    esum = spool.tile([P, 1], f32, tag="esum")
                    etmp = spool.tile([P, E], f32, tag="etmp")
                    nc.scalar.activation(out=etmp[:], in_=plg[:], func=ACT.Exp,
                                         bias=negm[:], scale=1.0,
                                         accum_out=esum[:])
                    g_t = spool.tile([P, 1], f32, tag="gate")
                    nc.vector.reciprocal(out=g_t[:], in_=esum[:])
                    gate.append(g_t)
                    nc.vector.tensor_scalar(
                        out=oh_all[:, t * E:(t + 1) * E], in0=plg[:],
                        scalar1=m_t[:], scalar2=None, op0=OP.is_equal)
                    if prescale:
                        xs_t = rpool.tile([P, H], f32, tag="xsc")
                        nc.vector.tensor_scalar(out=xs_t[:], in0=xtm[t][:],
                                                scalar1=g_t[:], scalar2=None,
                                                op0=OP.mult)
                        xsc.append(xs_t)
                    else:
                        xsc.append(xtm[t])
                        nc.scalar.dma_start(
                            out=gate_dram[t * P:(t + 1) * P, :], in_=g_t[:])

                # per-tile incremental slot computation; tile t's dispatch
                # scatter fires as soon as its softmax + running prefix land
                tbacc = spool.tile([P, E], f32, tag="tbacc")
                nc.vector.tensor_copy(out=tbacc[:], in_=ecm1_i[:])
                slot_is = []
                for t in range(NTT):
                    blk = slice(t * E, (t + 1) * E)
                    pcc = cpsum1.tile([P, 2 * E], f32, tag="pcc",
                                      space="PSUM")
                    nc.tensor.matmul(out=pcc[:, 0:E], lhsT=lt128[:],
                                     rhs=oh_all[:, blk], start=True,
                                     stop=True)
                    nc.tensor.matmul(out=pcc[:, E:2 * E], lhsT=ones128[:],
                                     rhs=oh_all[:, blk], start=True,
                                     stop=True)
                    tmp = spool.tile([P, E], f32, tag="tmp")
                    nc.vector.tensor_tensor(out=tmp[:], in0=pcc[:, 0:E],
                                            in1=tbacc[:], op=OP.add)
                    junk = spool.tile([P, E], f32, tag="junk")
                    nc.vector.tensor_tensor(out=junk[:], in0=tmp[:],
                                            in1=oh_all[:, blk], op=OP.mult)
                    slot_f = spool.tile([P, 1], f32, tag="slotf")
                    nc.vector.tensor_reduce(out=slot_f[:], in_=junk[:],
                                            axis=AX.X, op=OP.add)
                    slot_i = spool.tile([P, 1], i32, tag="sloti")
                    nc.vector.tensor_copy(out=slot_i[:], in_=slot_f[:])
                    slot_is.append(slot_i)
                    if t < NTT - 1:
                        nc.vector.tensor_tensor(out=tbacc[:], in0=tbacc[:],
                                                in1=pcc[:, E:2 * E],
                                                op=OP.add)
                    nc.gpsimd.indirect_dma_start(
                        out=xs_dram[:],
                        out_offset=bass.IndirectOffsetOnAxis(
                            ap=slot_i[:, :1], axis=0),
                        in_=xsc[t][:], in_offset=None)
                for t in range(NTT):
                    nc.gpsimd.indirect_dma_start(
                        out=gidx_dram[:],
                        out_offset=bass.IndirectOffsetOnAxis(
                            ap=slot_is[t][:, :1], axis=0),
                        in_=tid_all[:, t:t + 1], in_offset=None)

            # ---------------- phase 2: per-expert grouped GEMM ------------
            with (
                tc.tile_pool(name="est", bufs=3) as stpool,
                tc.tile_pool(name="exs", bufs=2 * KC) as xspool,
                tc.tile_pool(name="eyt", bufs=2 * MC) as ytpool,
                tc.tile_pool(name="eysb", bufs=3) as ypool,
                tc.tile_pool(name="egi", bufs=E) as gipool,
                tc.tile_pool(name="exps", bufs=2, space="PSUM") as xpsum,
                tc.tile_pool(name="eyps", bufs=2, space="PSUM") as ypsum,
                tc.tile_pool(name="etps", bufs=2, space="PSUM") as tpsum,
            ):
                # front-load all index readbacks (gpsimd queue, after
                # scatters; only needed at combine time)
                gAB = []
                for e in range(E):
                    base = e * CAP
                    gA = gipool.tile([CAPA, 1], i32, tag="gA")
                    nc.gpsimd.dma_start(out=gA[:],
                                        in_=gidx_dram[base:base + CAPA, :])
                    gB = gipool.tile([CAPB, 1], i32, tag="gB")
                    nc.gpsimd.dma_start(
                        out=gB[:], in_=gidx_dram[base + CAPA:base + CAP, :])
                    gAB.append((gA, gB))

                for e in range(E):
                    gA, gB = gAB[e]
                    base = e * CAP
                    stA = stpool.tile([CAPA, H], f32, tag="stA")
                    nc.scalar.dma_start(out=stA[:],
                                        in_=xs_dram[base:base + CAPA, :])
                    stB = stpool.tile([CAPB, H], f32, tag="stB")
                    nc.gpsimd.dma_start(
                        out=stB[:], in_=xs_dram[base + CAPA:base + CAP, :])
                    if e + 2 < E:
                        w_tiles[e + 2] = load_w(e + 2)

                    # fp32 transpose to feature-major, then hi/lo split on
                    # the (idle) vector engine
                    if hilo:
                        xsh, xsl = [], []
                        for k in range(KC):
                            ks = slice(k * P, (k + 1) * P)
                            pxs = xpsum.tile([P, CAP], f32, tag="pxs",
                                             space="PSUM")
                            nc.tensor.transpose(out=pxs[:, 0:CAPA],
                                                in_=stA[:, ks],
                                                identity=id128[:])
                            nc.tensor.transpose(out=pxs[:, CAPA:CAP],
                                                in_=stB[:, ks],
                                                identity=id128[:CAPB, :CAPB])
                            xshk = xspool.tile([P, CAP], bf16, tag="xsh")
                            nc.vector.tensor_copy(out=xshk[:], in_=pxs[:])
                            xshf = xspool.tile([P, CAP], f32, tag="xshf")
                            nc.vector.tensor_copy(out=xshf[:], in_=xshk[:])
                            xslk = xspool.tile([P, CAP], bf16, tag="xsl")
                            nc.vector.tensor_tensor(out=xslk[:], in0=pxs[:],
                                                    in1=xshf[:],
                                                    op=OP.subtract)
                            xsh.append(xshk)
                            xsl.append(xslk)
                        wh_sb, wl_sb = w_tiles[e]
                    else:
                        xs = []
                        for k in range(KC):
                            ks = slice(k * P, (k + 1) * P)
                            pxs = xpsum.tile([P, CAP], f32, tag="pxs",
                                             space="PSUM")
                            nc.tensor.transpose(out=pxs[:, 0:CAPA],
                                                in_=stA[:, ks],
                                                identity=id128[:])
                            nc.tensor.transpose(out=pxs[:, CAPA:CAP],
                                                in_=stB[:, ks],
                                                identity=id128[:CAPB, :CAPB])
                            xsk = xspool.tile([P, CAP], f32, tag="xs")
                            nc.vector.tensor_copy(out=xsk[:], in_=pxs[:])
                            xs.append(xsk)
                        w_sb = w_tiles[e]

                    if expert_bias:
                        eb_sb = gipool.tile([1, H], f32, tag="eb")
                        nc.scalar.dma_start(out=eb_sb[:], in_=eb_d[e, None, :])

                    yt = []
                    for m in range(MC):
                        ms = slice(m * P, (m + 1) * P)
                        pyt = ypsum.tile([P, CAP], f32, tag="pyt",
                                         space="PSUM")
                        if hilo:
                            for k in range(KC):
                                last = (k == KC - 1 and not expert_bias)
                                nc.tensor.matmul(
                                    out=pyt[:], lhsT=wh_sb[k][:, ms],
                                    rhs=xsh[k][:], start=(k == 0), stop=False)
                                nc.tensor.matmul(
                                    out=pyt[:], lhsT=wh_sb[k][:, ms],
                                    rhs=xsl[k][:], start=False, stop=False)
                                nc.tensor.matmul(
                                    out=pyt[:], lhsT=wl_sb[k][:, ms],
                                    rhs=xsh[k][:], start=False,
                                    stop=(last and not four_term))
                                if four_term:
                                    nc.tensor.matmul(
                                        out=pyt[:], lhsT=wl_sb[k][:, ms],
                                        rhs=xsl[k][:], start=False, stop=last)
                        else:
                            for k in range(KC):
                                nc.tensor.matmul(
                                    out=pyt[:], lhsT=w_sb[k][:, ms],
                                    rhs=xs[k][:], start=(k == 0),
                                    stop=(k == KC - 1 and not expert_bias))
                        if expert_bias:
                            nc.tensor.matmul(
                                out=pyt[:], lhsT=eb_sb[:, ms],
                                rhs=ones_cap[:], start=False, stop=True)
                        ytm = ytpool.tile([P, CAP], f32, tag="yt")
                        nc.vector.tensor_copy(out=ytm[:], in_=pyt[:])
                        yt.append(ytm)

                    if not prescale:
                        gsA = gipool.tile([CAPA, 1], f32, tag="gsA")
                        nc.gpsimd.indirect_dma_start(
                            out=gsA[:], out_offset=None, in_=gate_dram[:],
                            in_offset=bass.IndirectOffsetOnAxis(ap=gA[:, :1],
                                                                axis=0),
                            bounds_check=TPC - 1, oob_is_err=False)
                        gsB = gipool.tile([CAPB, 1], f32, tag="gsB")
                        nc.gpsimd.indirect_dma_start(
                            out=gsB[:], out_offset=None, in_=gate_dram[:],
                            in_offset=bass.IndirectOffsetOnAxis(ap=gB[:, :1],
                                                                axis=0),
                            bounds_check=TPC - 1, oob_is_err=False)

                    ptokA = tpsum.tile([P, H], f32, tag="ptok", space="PSUM")
                    for m in range(MC):
                        ms = slice(m * P, (m + 1) * P)
                        nc.tensor.transpose(out=ptokA[:, ms],
                                            in_=yt[m][:, 0:CAPA],
                                            identity=id128[:])
                    yA = ypool.tile([CAPA, H], f32, tag="yA")
                    if prescale:
                        nc.vector.tensor_copy(out=yA[:], in_=ptokA[:])
                    else:
                        nc.vector.tensor_scalar(out=yA[:], in0=ptokA[:],
                                                scalar1=gsA[:], scalar2=None,
                                                op0=OP.mult)
                    nc.gpsimd.indirect_dma_start(
                        out=y_d[:],
                        out_offset=bass.IndirectOffsetOnAxis(ap=gA[:, :1],
                                                            axis=0),
                        in_=yA[:], in_offset=None,
                        bounds_check=TPC - 1, oob_is_err=False)

                    ptokB = tpsum.tile([P, H], f32, tag="ptok", space="PSUM")
                    for m in range(MC):
                        ms = slice(m * P, (m + 1) * P)
                        nc.tensor.transpose(out=ptokB[0:CAPB, ms],
                                            in_=yt[m][:, CAPA:CAP],
                                            identity=id128[:])
                    yB = ypool.tile([CAPB, H], f32, tag="yB")
                    if prescale:
                        nc.vector.tensor_copy(out=yB[:], in_=ptokB[0:CAPB, :])
                    else:
                        nc.vector.tensor_scalar(out=yB[:], in0=ptokB[0:CAPB, :],
                                                scalar1=gsB[:], scalar2=None,
                                                op0=OP.mult)
                    nc.gpsimd.indirect_dma_start(
                        out=y_d[:],
                        out_offset=bass.IndirectOffsetOnAxis(ap=gB[:, :1],
                                                            axis=0),
                        in_=yB[:], in_offset=None,
                        bounds_check=TPC - 1, oob_is_err=False)

    nc.compile()
    return nc


_NC_CACHE = {}


def _get_nc(router_bias: bool, expert_bias: bool, cap: int = DEFAULT_CAP,
            prec: str = PREC):
    key = (router_bias, expert_bias, cap, prec)
    if key not in _NC_CACHE:
        _NC_CACHE[key] = _build(*key)
    return _NC_CACHE[key]


def _split_hilo(w):
    import ml_dtypes
    hi = w.astype(ml_dtypes.bfloat16)
    lo = (w - hi.astype(np.float32)).astype(ml_dtypes.bfloat16)
    return np.ascontiguousarray(hi), np.ascontiguousarray(lo)


def balanced_perm(eidx):
    """Token permutation dealing each expert's tokens across cores so every
    (core, expert) group is ~n_e/NCORES and core totals are exactly TPC."""
    T = eidx.shape[0]
    groups = [np.where(eidx == e)[0] for e in range(E)]
    counts = np.zeros((NCORES, E), dtype=np.int64)
    for e in range(E):
        n = len(groups[e])
        base, rem = divmod(n, NCORES)
        counts[:, e] = base
        # give the remainder to the currently least-loaded cores
        order = np.argsort(counts.sum(1), kind="stable")
        counts[order[:rem], e] += 1
    # fix core totals to exactly TPC by moving single tokens
    totals = counts.sum(1)
    while True:
        hi_c = int(np.argmax(totals))
        lo_c = int(np.argmin(totals))
        if totals[hi_c] <= TPC and totals[lo_c] >= TPC:
            break
        moved = False
        for e in np.argsort(-counts[hi_c]):
            if counts[hi_c, e] > 0:
                counts[hi_c, e] -= 1
                counts[lo_c, e] += 1
                totals[hi_c] -= 1
                totals[lo_c] += 1
                moved = True
                break
        assert moved
    assert (counts.sum(1) == TPC).all()
    # build per-core token lists following the counts
    taken = [0] * E
    core_tokens = []
    for c in range(NCORES):
        toks = []
        for e in range(E):
            k = counts[c, e]
            toks.append(groups[e][taken[e]:taken[e] + k])
            taken[e] += k
        core_tokens.append(np.concatenate(toks))
    perm = np.concatenate(core_tokens)
    assert perm.shape == (T,) and len(np.unique(perm)) == T
    return perm, int(counts.max())


def plan(x, router_w, router_b):
    """Host-side shard plan: balanced permutation + capacity."""
    logits = x.reshape(-1, H) @ router_w + router_b
    eidx = logits.argmax(-1)
    perm, maxcell = balanced_perm(eidx)
    cap = max(DEFAULT_CAP, ((maxcell + 7) // 8) * 8)
    return perm, cap


def make_in_maps(x, router_w, router_b, expert_w, expert_b, perm,
                 prec=PREC):
    xt = x.reshape(-1, H)[perm].reshape(NCORES, TPC, H)
    base = {"router_w": router_w, "router_b": router_b, "expert_b": expert_b}
    if prec.startswith("hilo"):
        hi, lo = _split_hilo(expert_w)
        base["ew_hi"] = hi
        base["ew_lo"] = lo
    else:
        base["expert_w"] = expert_w
    return [dict(base, x=np.ascontiguousarray(xt[c])) for c in range(NCORES)]


def kernel(x, router_w, router_b, expert_w, expert_b):
    from concourse.bass_utils import run_bass_kernel_spmd

    x = np.ascontiguousarray(np.asarray(x, dtype=np.float32))
    router_w = np.ascontiguousarray(np.asarray(router_w, dtype=np.float32))
    router_b = np.ascontiguousarray(np.asarray(router_b, dtype=np.float32))
    expert_w = np.ascontiguousarray(np.asarray(expert_w, dtype=np.float32))
    expert_b = np.ascontiguousarray(np.asarray(expert_b, dtype=np.float32))

    B, S, Hx = x.shape
    assert (B * S, Hx) == (NCORES * TPC, H), (x.shape,)

    perm, cap = plan(x, router_w, router_b)
    router_bias = bool(np.any(router_b != 0))
    expert_bias = bool(np.any(expert_b != 0))
    nc = _get_nc(router_bias, expert_bias, cap)

    in_maps = make_in_maps(x, router_w, router_b, expert_w, expert_b, perm)
    res = run_bass_kernel_spmd(nc, in_maps, list(range(NCORES)))
    y_perm = np.concatenate([res.results[c]["y"] for c in range(NCORES)],
                            axis=0)
    y = np.empty_like(y_perm)
    y[perm] = y_perm
    return y.reshape(B, S, H)
